# revision 1
# baseline (speedup 1.0000x reference)
"""Trainium2 Bass kernel for ConstructAdjMatrixWithHomogeneous.

out = I + D^-1/2 @ adj @ D^-1/2,  adj = [[C, A], [A^T, Dd]],
C = filtered_cell_kernel [4000,4000], Dd = filtered_drug_sim [4000,4000],
A = original_cell_drug_adj [4000,4000]; deg = rowsum(adj)+eps, d = deg**-0.5.

Sharding (8 cores): overlapping 512-row slices of each input matrix —
row starts R0 = [0, 512, ..., 3072, 3488]; core 7 overlaps core 6 by 96
rows so every slice is exactly 512 = 4x128 rows (128-partition DMA tiles
run ~3.5x faster than partial-partition tiles on this part). Core 7's A
slice has its 96 overlap rows zeroed host-side so the column-sum partial
is not double counted; all other overlap outputs are simply discarded at
assembly.

Launch 1: row sums of C/A/Dd bands (DVE reduce) + partial column sums of
A (PE ones-matmul into PSUM). Host gathers the 8000-long degree vector
(the "all-gather"), computes d = rsqrt(deg+eps).
Launch 2: row scale (ACT, per-partition scale) + column scale (DVE mul
with broadcast d) of each band; writes top rows [512,8000] and
bottom-right rows [512,4000]; the bottom-left block A^T is produced by
PE-transposing the scaled A tiles and written as a column slab [4000,512].
The +1 identity is folded into the inputs host-side: adding (deg_i+eps)
to adj[i,i] makes d_i*(adj_ii + deg_i+eps)*d_i == d_i*adj_ii*d_i + 1.

DMA discipline learned from microbenchmarks on this setup: HWDGE only
(SWDGE is broken in this walrus build), loads and dependent stores on
separate HWDGE rings (sync vs scalar) so the per-ring FIFO never stalls
a load behind a store that waits on compute.
"""
import sys

sys.path.insert(0, "/opt/trn_rl_repo")

import contextlib
import json
import numpy as np

import concourse.bass as bass
import concourse.mybir as mybir
import concourse.tile as tile
import concourse.bass2jax as bass2jax
from concourse.bass_utils import run_bass_kernel_spmd, compile_bir_kernel

F32 = mybir.dt.float32
NCORES = 8
PB = 128               # partition band size
NBAND = 4
CR = PB * NBAND        # 512 rows of each matrix per core (overlapping)
NMAT = 4000
N = 8000
EPS = np.float32(1e-9)
R0 = [512 * k for k in range(7)] + [NMAT - CR]          # slice starts
OWN = [(512 * k, 512 * (k + 1)) for k in range(7)] + [(3584, 4000)]

# ---------------------------------------------------------------------------
# Walrus workaround: this toolchain only supports ONE sync-wait condition per
# instruction ("Too many sync wait commands" in CoreV3GenImpl otherwise).
# Split any instruction carrying >1 waits into preceding NoOps, 1 wait each.
# ---------------------------------------------------------------------------
_MAXW = 1


def _split_waits_bytes(bir_bytes):
    bir = json.loads(bir_bytes)
    n_new = 0
    for fn in bir["functions"]:
        for blk in fn["blocks"]:
            insts = blk.get("instructions", [])
            out = []
            for ins in insts:
                si = ins.get("sync_info") or {}
                waits = si.get("on_wait") or []
                while len(waits) > _MAXW:
                    chunk, waits = waits[:_MAXW], waits[_MAXW:]
                    n_new += 1
                    out.append({
                        "name": ins["name"] + f"_ws{n_new}",
                        "opcode": "NoOp",
                        "engine": ins["engine"],
                        "ins": [], "outs": [],
                        "sync_info": {"on_update": [], "on_wait": chunk},
                    })
                si["on_wait"] = waits
                ins["sync_info"] = si
                out.append(ins)
            blk["instructions"] = out
    return json.dumps(bir).encode()


def _patched_compile_bir_kernel(bir_json, tmpdir, neff_name="file.neff"):
    return compile_bir_kernel(_split_waits_bytes(bir_json), tmpdir,
                              neff_name=neff_name)


bass2jax.compile_bir_kernel = _patched_compile_bir_kernel


def _rep_ctx(tc, reps):
    # reps>1 is a timing-only mode: run the body in a hardware loop.
    return tc.For_i(0, reps, 1) if reps > 1 else contextlib.nullcontext()


# ---------------------------------------------------------------------------
# Launch 1: degree partials.
#   rs_c/rs_a/rs_d [512,1] row sums of this core's C/A/Dd rows,
#   cs_a [1,4000] partial column sums of this core's A rows.
# ---------------------------------------------------------------------------
def _build_l1(reps=1):
    nc = bass.Bass()
    cb = nc.declare_dram_parameter("cb", [CR, NMAT], F32, isOutput=False)
    ab = nc.declare_dram_parameter("ab", [CR, NMAT], F32, isOutput=False)
    db = nc.declare_dram_parameter("db", [CR, NMAT], F32, isOutput=False)
    rs_c = nc.declare_dram_parameter("rs_c", [CR, 1], F32, isOutput=True)
    rs_a = nc.declare_dram_parameter("rs_a", [CR, 1], F32, isOutput=True)
    rs_d = nc.declare_dram_parameter("rs_d", [CR, 1], F32, isOutput=True)
    cs_a = nc.declare_dram_parameter("cs_a", [1, NMAT], F32, isOutput=True)

    NCHUNK = 8
    CW = NMAT // NCHUNK  # 500

    with tile.TileContext(nc) as tc:
        with (
            tc.tile_pool(name="inp", bufs=4) as inp,
            tc.tile_pool(name="red", bufs=8) as red,
            tc.tile_pool(name="csout", bufs=2) as csout,
            tc.tile_pool(name="const", bufs=1) as const,
            tc.tile_pool(name="ps", bufs=1, space="PSUM") as ps,
        ):
            ones = const.tile([PB, 1], F32)
            nc.gpsimd.memset(ones[:], 1.0)

            pscs = [ps.tile([1, CW], F32, tag=f"cs{j}", name=f"cs{j}")
                    for j in range(NCHUNK)]

            with _rep_ctx(tc, reps):
                for src, rsout, is_a in ((cb, rs_c, False), (ab, rs_a, True),
                                         (db, rs_d, False)):
                    for b in range(NBAND):
                        t = inp.tile([PB, NMAT], F32, tag="t", name="t")
                        nc.sync.dma_start(t[:], src[b * PB:(b + 1) * PB, :])
                        r = red.tile([PB, 1], F32, tag="r", name="r")
                        nc.vector.reduce_sum(r[:], t[:],
                                             axis=mybir.AxisListType.X)
                        nc.scalar.dma_start(rsout[b * PB:(b + 1) * PB, :], r[:])
                        if is_a:
                            for j in range(NCHUNK):
                                nc.tensor.matmul(
                                    pscs[j][:],
                                    ones[:],
                                    t[:, j * CW:(j + 1) * CW],
                                    start=(b == 0),
                                    stop=(b == NBAND - 1),
                                )
                for j in range(NCHUNK):
                    cst = csout.tile([1, CW], F32, tag="cs", name="cst")
                    nc.scalar.copy(cst[:], pscs[j][:])
                    nc.scalar.dma_start(cs_a[0:1, j * CW:(j + 1) * CW], cst[:])
    return nc


# ---------------------------------------------------------------------------
# Launch 2: scaling + assembly.
# Inputs: cb/ab/db [512,4000] (cb/db carry the host-folded diagonal fix,
#   ab zeroed overlap rows on core 7), drow [128,8] (col b = d of cell band
#   b rows, col 4+b = d of drug band b rows), dbc [128,8000] (d broadcast).
# Outputs: top [512,8000], br [512,4000], ats [4000,512].
# ---------------------------------------------------------------------------
def _build_l2(reps=1):
    nc = bass.Bass()
    cb = nc.declare_dram_parameter("cb", [CR, NMAT], F32, isOutput=False)
    ab = nc.declare_dram_parameter("ab", [CR, NMAT], F32, isOutput=False)
    db = nc.declare_dram_parameter("db", [CR, NMAT], F32, isOutput=False)
    drow = nc.declare_dram_parameter("drow", [PB, 2 * NBAND], F32, isOutput=False)
    dbc = nc.declare_dram_parameter("dbc", [PB, N], F32, isOutput=False)
    top = nc.declare_dram_parameter("top", [CR, N], F32, isOutput=True)
    br = nc.declare_dram_parameter("br", [CR, NMAT], F32, isOutput=True)
    ats = nc.declare_dram_parameter("ats", [NMAT, CR], F32, isOutput=True)

    ident = nc.inline_tensor(np.eye(PB, dtype=np.float32), name="ident128")

    Copy = mybir.ActivationFunctionType.Copy
    NFULL = NMAT // PB  # 31 full transpose chunks
    TAIL = NMAT - NFULL * PB  # 32

    with tile.TileContext(nc) as tc:
        with (
            tc.tile_pool(name="const", bufs=1) as const,
            tc.tile_pool(name="inp", bufs=3) as inp,
            tc.tile_pool(name="outs", bufs=2) as outs,
            tc.tile_pool(name="ascl", bufs=1) as ascl,
            tc.tile_pool(name="att", bufs=3) as att,
            tc.tile_pool(name="pst", bufs=4, space="PSUM") as pst,
        ):
            dbct = const.tile([PB, N], F32)
            nc.sync.dma_start(dbct[:], dbc[:])
            drt = const.tile([PB, 2 * NBAND], F32)
            nc.sync.dma_start(drt[:], drow[:])
            idt = const.tile([PB, PB], F32)
            nc.sync.dma_start(idt[:], ident[:])

            with _rep_ctx(tc, reps):
                # --- A rows first (frees the transpose tail to overlap C/D) ---
                a_scaled = []
                for b in range(NBAND):
                    ain = inp.tile([PB, NMAT], F32, tag="inp", name="ain")
                    nc.sync.dma_start(ain[:], ab[b * PB:(b + 1) * PB, :])
                    at = ascl.tile([PB, NMAT], F32, tag=f"as{b}", name="at")
                    nc.scalar.activation(at[:], ain[:], Copy,
                                         scale=drt[:, b:b + 1])
                    nc.vector.tensor_mul(at[:], at[:], dbct[:, NMAT:])
                    nc.scalar.dma_start(top[b * PB:(b + 1) * PB, NMAT:], at[:])
                    a_scaled.append(at)

                # --- transposed A slab ---
                for c in range(NFULL + 1):
                    cw = PB if c < NFULL else TAIL
                    pt = pst.tile([cw, CR], F32, tag="pt", name="pt")
                    for b in range(NBAND):
                        nc.tensor.transpose(
                            pt[:, b * PB:(b + 1) * PB],
                            a_scaled[b][:, c * PB:c * PB + cw],
                            idt[:],
                        )
                    at_sb = att.tile([cw, CR], F32, tag="att", name="at_sb")
                    nc.scalar.copy(at_sb[:], pt[:])
                    nc.scalar.dma_start(ats[c * PB:c * PB + cw, :], at_sb[:])

                # --- C and D rows ---
                for b in range(NBAND):
                    cin = inp.tile([PB, NMAT], F32, tag="inp", name="cin")
                    nc.sync.dma_start(cin[:], cb[b * PB:(b + 1) * PB, :])
                    ct = outs.tile([PB, NMAT], F32, tag="outs", name="ct")
                    nc.scalar.activation(ct[:], cin[:], Copy,
                                         scale=drt[:, b:b + 1])
                    nc.vector.tensor_mul(ct[:], ct[:], dbct[:, 0:NMAT])
                    nc.scalar.dma_start(top[b * PB:(b + 1) * PB, 0:NMAT], ct[:])

                    din = inp.tile([PB, NMAT], F32, tag="inp", name="din")
                    nc.sync.dma_start(din[:], db[b * PB:(b + 1) * PB, :])
                    dt = outs.tile([PB, NMAT], F32, tag="outs", name="dt")
                    nc.scalar.activation(dt[:], din[:], Copy,
                                         scale=drt[:, NBAND + b:NBAND + b + 1])
                    nc.vector.tensor_mul(dt[:], dt[:], dbct[:, NMAT:])
                    nc.scalar.dma_start(br[b * PB:(b + 1) * PB, :], dt[:])
    return nc


_programs_cache = {}


def _programs():
    if "l1" not in _programs_cache:
        _programs_cache["l1"] = _build_l1()
        _programs_cache["l2"] = _build_l2()
    return _programs_cache["l1"], _programs_cache["l2"]


def kernel(filtered_cell_kernel, filtered_drug_sim, original_cell_drug_adj,
           enable_homogeneous_graph):
    C = np.ascontiguousarray(np.asarray(filtered_cell_kernel, dtype=np.float32))
    D = np.ascontiguousarray(np.asarray(filtered_drug_sim, dtype=np.float32))
    A = np.ascontiguousarray(np.asarray(original_cell_drug_adj, dtype=np.float32))
    enable = int(np.asarray(enable_homogeneous_graph))
    if not enable:
        C = np.zeros_like(C)
        D = np.zeros_like(D)

    l1, l2 = _programs()
    cores = list(range(NCORES))

    Cb = [C[R0[k]:R0[k] + CR] for k in range(NCORES)]
    Db = [D[R0[k]:R0[k] + CR] for k in range(NCORES)]
    Ab = [A[R0[k]:R0[k] + CR] for k in range(NCORES)]
    ab7 = Ab[7].copy()
    ab7[: OWN[7][0] - R0[7]] = 0.0   # zero the 96 overlap rows
    Ab[7] = ab7

    in1 = [{"cb": Cb[k], "ab": Ab[k], "db": Db[k]} for k in range(NCORES)]
    r1 = run_bass_kernel_spmd(l1, in1, core_ids=cores).results

    deg = np.empty(N, dtype=np.float32)
    cs_a = np.zeros(NMAT, dtype=np.float32)
    for k in range(NCORES):
        s, e = OWN[k]
        lo = s - R0[k]
        deg[s:e] = (r1[k]["rs_c"][lo:lo + (e - s), 0]
                    + r1[k]["rs_a"][lo:lo + (e - s), 0])
        deg[NMAT + s:NMAT + e] = r1[k]["rs_d"][lo:lo + (e - s), 0]
        cs_a += r1[k]["cs_a"][0]
    deg[NMAT:] += cs_a

    total = float(deg.astype(np.float64).sum())
    if total == 0.0:
        return np.eye(N, dtype=np.float32)

    degp = (deg + EPS).astype(np.float32)
    d = degp ** np.float32(-0.5)
    d = np.where(np.isinf(d), np.float32(0.0), d).astype(np.float32)

    dbc = np.ascontiguousarray(np.broadcast_to(d, (PB, N)))
    idx = np.arange(CR)
    in2 = []
    for k in range(NCORES):
        r0 = R0[k]
        cbk = Cb[k].copy()
        cbk[idx, r0 + idx] += degp[r0 + idx]
        dbk = Db[k].copy()
        dbk[idx, r0 + idx] += degp[NMAT + r0 + idx]
        drow_k = np.concatenate([d[r0:r0 + CR], d[NMAT + r0:NMAT + r0 + CR]])
        drow = np.ascontiguousarray(drow_k.reshape(2 * NBAND, PB).T)
        in2.append({"cb": cbk, "ab": Ab[k], "db": dbk,
                    "drow": drow, "dbc": dbc})

    r2 = run_bass_kernel_spmd(l2, in2, core_ids=cores).results

    out = np.empty((N, N), dtype=np.float32)
    for k in range(NCORES):
        s, e = OWN[k]
        lo = s - R0[k]
        out[s:e, :] = r2[k]["top"][lo:lo + (e - s)]
        out[NMAT + s:NMAT + e, NMAT:] = r2[k]["br"][lo:lo + (e - s)]
        out[NMAT:, s:e] = r2[k]["ats"][:, lo:lo + (e - s)]
    return out



# revision 5
# speedup vs baseline: 1.3312x; 1.3312x over previous
"""Trainium2 Bass kernel for ConstructAdjMatrixWithHomogeneous — single launch.

out = I + D^-1/2 @ adj @ D^-1/2,  adj = [[C, A], [A^T, Dd]],
C = filtered_cell_kernel [4000,4000], Dd = filtered_drug_sim [4000,4000],
A = original_cell_drug_adj [4000,4000]; deg = rowsum(adj)+eps, d = deg**-0.5.

Sharding (8 cores): 128-aligned row bands. Core c owns rows
[512c, 512c+512) of each matrix; core 7 owns rows 3584:4000 plus 96
zero-padded junk rows so every DMA tile is a full [128,4000]
(partial-partition DMAs measured 3x slower per byte on this part).

Single launch per core:
  Phase A — load A, C, D bands into SBUF (12 x [128,4000] tiles stay
  resident, 187.5 KiB/partition). Cell degrees are purely local
  (rowsum C + rowsum A). The only cross-core reduction is the drug-side
  column sum of A: partial colsums go through a ReduceScatter(add) so
  each core gets exactly its own 512-row chunk back. Two tiny
  AllGathers then distribute the locally-computed d = (deg+eps)^-0.5
  values: AG1 (cell d, fired as soon as A+C are loaded) and AG2
  (drug d, after D rowsums + RS result). All collective payloads are
  2-16KB DRAM bounce buffers.
  Phase B — scale in place with one fused DVE op per 512-col chunk:
  scalar_tensor_tensor(x, x, drow, bc) = (x * d_row) * d_col, where bc
  is a [128,512] PSUM broadcast of the gathered d line chunk (ones-row
  matmul). The top-left block only needs cell d, so its stores overlap
  the tail of the D loads. The bottom-left block A'^T is produced by
  PE-transposing the scaled A tiles.

Ring discipline (measured): one HWDGE ring sustains ~317 GB/s, two
directions on separate rings ~420 GB/s aggregate, mixed directions on
a ring much worse. Loads ride sync; TL/TR stores ride scalar; BR/ats
stores ride sync after the loads drain. d-chunk loads go on whichever
ring is idle at that point (cell: scalar, drug: sync).

rel-err vs fp64 reference ~1e-6 (PE fp32 matmuls are exact here; DVE
reciprocal + ACT Sqrt for rsqrt since ACT Rsqrt is banned).
The +I is added on the host during assembly (O(N) work).
"""
import sys

sys.path.insert(0, "/opt/trn_rl_repo")

import json
import numpy as np

import concourse.bass as bass
import concourse.mybir as mybir
import concourse.tile as tile
import concourse.bass2jax as bass2jax
from concourse.alu_op_type import AluOpType
from concourse.bass_utils import run_bass_kernel_spmd, compile_bir_kernel

F32 = mybir.dt.float32
NCORES = 8
PB = 128               # partition band size
NBAND = 4
CR = PB * NBAND        # 512 rows of each matrix per core
NMAT = 4000
N = 8000
EPS = np.float32(1e-9)
NCH = 8                # 512-wide column chunks; last is 416
CHW = [512] * 7 + [416]
NFULL = NMAT // PB     # 31 full transpose chunks
TAIL = NMAT - NFULL * PB  # 32

# ---------------------------------------------------------------------------
# Walrus workaround: this toolchain only supports ONE sync-wait condition per
# instruction ("Too many sync wait commands" in CoreV3GenImpl otherwise).
# Split any instruction carrying >1 waits into preceding NoOps, 1 wait each.
# ---------------------------------------------------------------------------
_MAXW = 1


def _split_waits_bytes(bir_bytes):
    bir = json.loads(bir_bytes)
    n_new = 0
    for fn in bir["functions"]:
        for blk in fn["blocks"]:
            insts = blk.get("instructions", [])
            out = []
            for ins in insts:
                si = ins.get("sync_info") or {}
                waits = si.get("on_wait") or []
                while len(waits) > _MAXW:
                    chunk, waits = waits[:_MAXW], waits[_MAXW:]
                    n_new += 1
                    out.append({
                        "name": ins["name"] + f"_ws{n_new}",
                        "opcode": "NoOp",
                        "engine": ins["engine"],
                        "ins": [], "outs": [],
                        "sync_info": {"on_update": [], "on_wait": chunk},
                    })
                si["on_wait"] = waits
                ins["sync_info"] = si
                out.append(ins)
            blk["instructions"] = out
    return json.dumps(bir).encode()


def _patched_compile_bir_kernel(bir_json, tmpdir, neff_name="file.neff"):
    return compile_bir_kernel(_split_waits_bytes(bir_json), tmpdir,
                              neff_name=neff_name)


bass2jax.compile_bir_kernel = _patched_compile_bir_kernel


def _build(reps=1):
    nc = bass.Bass(num_devices=NCORES)
    cb = nc.declare_dram_parameter("cb", [CR, NMAT], F32, isOutput=False)
    ab = nc.declare_dram_parameter("ab", [CR, NMAT], F32, isOutput=False)
    db = nc.declare_dram_parameter("db", [CR, NMAT], F32, isOutput=False)
    top = nc.declare_dram_parameter("top", [CR, N], F32, isOutput=True)
    br = nc.declare_dram_parameter("br", [CR, NMAT], F32, isOutput=True)
    ats = nc.declare_dram_parameter("ats", [NMAT, CR], F32, isOutput=True)

    ident = nc.inline_tensor(np.eye(PB, dtype=np.float32), name="ident128")
    Sqrt = mybir.ActivationFunctionType.Sqrt
    RG = [list(range(NCORES))]

    with tile.TileContext(nc) as tc:
        with (
            tc.tile_pool(name="const", bufs=1) as const,
            tc.tile_pool(name="data", bufs=1) as data,
            tc.tile_pool(name="small", bufs=1) as small,
            tc.tile_pool(name="stg", bufs=2) as stg,
            tc.tile_pool(name="dstg", bufs=2) as dstg,
            tc.tile_pool(name="att", bufs=2) as att,
            tc.tile_pool(name="ps", bufs=1, space="PSUM") as ps,
            tc.tile_pool(name="dram", bufs=1, space="DRAM") as dram,
        ):
            idt = const.tile([PB, PB], F32, name="idt")
            nc.sync.dma_start(idt[:], ident[:])
            ones128 = const.tile([PB, 1], F32, name="ones128")
            nc.gpsimd.memset(ones128[:], 1.0)
            ones_row = const.tile([1, PB], F32, name="ones_row")
            nc.gpsimd.memset(ones_row[:], 1.0)
            ones1 = const.tile([1, 1], F32, name="ones1")
            nc.gpsimd.memset(ones1[:], 1.0)

            for _ in range(reps):
                rs_in = dram.tile([1, 4096], F32, tag="rs_in", name="rs_in")
                rs_out = dram.tile([1, 512], F32, tag="rs_out", name="rs_out")
                agc_in = dram.tile([1, 512], F32, tag="agc_in", name="agc_in")
                agc_out = dram.tile([1, 4096], F32, tag="agc_out", name="agc_out")
                agd_in = dram.tile([1, 512], F32, tag="agd_in", name="agd_in")
                agd_out = dram.tile([1, 4096], F32, tag="agd_out", name="agd_out")

                rs_a = small.tile([PB, NBAND], F32, tag="rs_a", name="rs_a")
                rs_c = small.tile([PB, NBAND], F32, tag="rs_c", name="rs_c")
                rs_d = small.tile([PB, NBAND], F32, tag="rs_d", name="rs_d")
                deg_c = small.tile([PB, NBAND], F32, tag="deg_c", name="deg_c")
                deg_d = small.tile([PB, NBAND], F32, tag="deg_d", name="deg_d")
                drow_c = small.tile([PB, NBAND], F32, tag="drow_c", name="drow_c")
                drow_d = small.tile([PB, NBAND], F32, tag="drow_d", name="drow_d")

                # ---- phase A: A bands -> rowsums + colsum partials ----
                cs_ps = [ps.tile([1, 512], F32, tag=f"ps{j}", name=f"cs{j}")
                         for j in range(NCH)]
                ta = []
                for b in range(NBAND):
                    t = data.tile([PB, NMAT], F32, tag=f"a{b}", name="ta")
                    nc.sync.dma_start(t[:], ab[b * PB:(b + 1) * PB, :])
                    nc.vector.reduce_sum(rs_a[:, b:b + 1], t[:],
                                         axis=mybir.AxisListType.X)
                    for j in range(NCH):
                        w = CHW[j]
                        nc.tensor.matmul(cs_ps[j][:, :w], ones128[:],
                                         t[:, 512 * j:512 * j + w],
                                         start=(b == 0), stop=(b == NBAND - 1))
                    ta.append(t)

                for j in range(NCH):
                    w = CHW[j]
                    cst = stg.tile([1, 512], F32, tag="cs_stg", name="cst")
                    nc.scalar.copy(cst[:, :w], cs_ps[j][:, :w])
                    nc.scalar.dma_start(rs_in[0:1, 512 * j:512 * j + w],
                                        cst[:, :w])
                nc.gpsimd.collective_compute(
                    "ReduceScatter", AluOpType.add, replica_groups=RG,
                    ins=[rs_in.opt()], outs=[rs_out.opt()])

                # ---- C bands -> rowsums -> cell d -> AllGather 1 ----
                tcl = []
                for b in range(NBAND):
                    t = data.tile([PB, NMAT], F32, tag=f"c{b}", name="tcl")
                    nc.sync.dma_start(t[:], cb[b * PB:(b + 1) * PB, :])
                    nc.vector.reduce_sum(rs_c[:, b:b + 1], t[:],
                                         axis=mybir.AxisListType.X)
                    tcl.append(t)
                nc.vector.tensor_add(deg_c[:], rs_c[:], rs_a[:])
                nc.vector.tensor_scalar_add(deg_c[:], deg_c[:], float(EPS))
                nc.vector.reciprocal(drow_c[:], deg_c[:])
                nc.scalar.activation(drow_c[:], drow_c[:], Sqrt)
                ptc = ps.tile([1, 512], F32, tag="ps0", name="ptc")
                for b in range(NBAND):
                    nc.tensor.transpose(ptc[0:1, PB * b:PB * (b + 1)],
                                        drow_c[:, b:b + 1], idt[:])
                agcs = stg.tile([1, 512], F32, tag="ag_stg", bufs=1, name="agcs")
                nc.scalar.copy(agcs[:], ptc[:])
                nc.scalar.dma_start(agc_in[:], agcs[:])
                nc.gpsimd.collective_compute(
                    "AllGather", AluOpType.bypass, replica_groups=RG,
                    ins=[agc_in.opt()], outs=[agc_out.opt()])

                # ---- D bands -> rowsums; drug d -> AllGather 2 ----
                td = []
                for b in range(NBAND):
                    t = data.tile([PB, NMAT], F32, tag=f"d{b}", name="td")
                    nc.sync.dma_start(t[:], db[b * PB:(b + 1) * PB, :])
                    nc.vector.reduce_sum(rs_d[:, b:b + 1], t[:],
                                         axis=mybir.AxisListType.X)
                    td.append(t)
                rso = stg.tile([1, 512], F32, tag="rso", bufs=1, name="rso")
                nc.scalar.dma_start(rso[:], rs_out[:])
                ptcs = ps.tile([PB, NBAND], F32, tag="ps1", name="ptcs")
                for b in range(NBAND):
                    nc.tensor.transpose(ptcs[:, b:b + 1],
                                        rso[0:1, PB * b:PB * (b + 1)],
                                        ones1[:])
                nc.vector.tensor_add(deg_d[:], rs_d[:], ptcs[:])
                nc.vector.tensor_scalar_add(deg_d[:], deg_d[:], float(EPS))
                nc.vector.reciprocal(drow_d[:], deg_d[:])
                nc.scalar.activation(drow_d[:], drow_d[:], Sqrt)
                ptd = ps.tile([1, 512], F32, tag="ps2", name="ptd")
                for b in range(NBAND):
                    nc.tensor.transpose(ptd[0:1, PB * b:PB * (b + 1)],
                                        drow_d[:, b:b + 1], idt[:])
                agds = stg.tile([1, 512], F32, tag="ag_stg", bufs=1, name="agds")
                nc.scalar.copy(agds[:], ptd[:])
                nc.scalar.dma_start(agd_in[:], agds[:])
                nc.gpsimd.collective_compute(
                    "AllGather", AluOpType.bypass, replica_groups=RG,
                    ins=[agd_in.opt()], outs=[agd_out.opt()])

                # ---- phase B: TL = dcell * C * dcell ----
                for j in range(NCH):
                    w = CHW[j]
                    dch = dstg.tile([1, 512], F32, tag="dch", name="dch")
                    nc.scalar.dma_start(dch[:, :w],
                                        agc_out[0:1, 512 * j:512 * j + w])
                    bc = ps.tile([PB, 512], F32, tag=f"ps{3 + (j % 2)}",
                                 name="bc")
                    nc.tensor.matmul(bc[:, :w], ones_row[:], dch[0:1, :w],
                                     start=True, stop=True)
                    for b in range(NBAND):
                        sl = slice(512 * j, 512 * j + w)
                        nc.vector.scalar_tensor_tensor(
                            tcl[b][:, sl], tcl[b][:, sl], drow_c[:, b:b + 1],
                            bc[:, :w], AluOpType.mult, AluOpType.mult)
                for b in range(NBAND):
                    nc.scalar.dma_start(top[b * PB:(b + 1) * PB, 0:NMAT],
                                        tcl[b][:])

                # ---- TR = dcell * A * ddrug, BR = ddrug * D * ddrug ----
                for j in range(NCH):
                    w = CHW[j]
                    dch = dstg.tile([1, 512], F32, tag="dch", name="dch")
                    nc.sync.dma_start(dch[:, :w],
                                      agd_out[0:1, 512 * j:512 * j + w])
                    bc = ps.tile([PB, 512], F32, tag=f"ps{5 + (j % 2)}",
                                 name="bcd")
                    nc.tensor.matmul(bc[:, :w], ones_row[:], dch[0:1, :w],
                                     start=True, stop=True)
                    for b in range(NBAND):
                        sl = slice(512 * j, 512 * j + w)
                        nc.vector.scalar_tensor_tensor(
                            ta[b][:, sl], ta[b][:, sl], drow_c[:, b:b + 1],
                            bc[:, :w], AluOpType.mult, AluOpType.mult)
                        nc.vector.scalar_tensor_tensor(
                            td[b][:, sl], td[b][:, sl], drow_d[:, b:b + 1],
                            bc[:, :w], AluOpType.mult, AluOpType.mult)
                for b in range(NBAND):
                    nc.scalar.dma_start(top[b * PB:(b + 1) * PB, NMAT:],
                                        ta[b][:])
                for b in range(NBAND):
                    nc.sync.dma_start(br[b * PB:(b + 1) * PB, :], td[b][:])

                # ---- ats = (scaled A)^T ----
                for c in range(NFULL + 1):
                    cw = PB if c < NFULL else TAIL
                    pt = ps.tile([PB, 512], F32, tag=f"ps{7 - (c % 3) * 2}",
                                 name="pt")
                    for b in range(NBAND):
                        nc.tensor.transpose(
                            pt[:cw, b * PB:(b + 1) * PB],
                            ta[b][:, c * PB:c * PB + cw], idt[:])
                    at_sb = att.tile([PB, 512], F32, tag="att", name="at_sb")
                    nc.scalar.copy(at_sb[:cw, :], pt[:cw, :])
                    nc.sync.dma_start(ats[c * PB:c * PB + cw, :],
                                      at_sb[:cw, :])
    return nc


_programs_cache = {}


def _program():
    if "l" not in _programs_cache:
        _programs_cache["l"] = _build()
    return _programs_cache["l"]


def _make_in_maps(C, A, D):
    in_maps = []
    for c in range(NCORES):
        s = 512 * c
        e = min(s + CR, NMAT)
        if e - s == CR:
            in_maps.append({"cb": C[s:e], "ab": A[s:e], "db": D[s:e]})
        else:
            m = {}
            for name, M in (("cb", C), ("ab", A), ("db", D)):
                t = np.zeros((CR, NMAT), dtype=np.float32)
                t[:e - s] = M[s:e]
                m[name] = t
            in_maps.append(m)
    return in_maps


def kernel(filtered_cell_kernel, filtered_drug_sim, original_cell_drug_adj,
           enable_homogeneous_graph):
    C = np.ascontiguousarray(np.asarray(filtered_cell_kernel, dtype=np.float32))
    D = np.ascontiguousarray(np.asarray(filtered_drug_sim, dtype=np.float32))
    A = np.ascontiguousarray(np.asarray(original_cell_drug_adj, dtype=np.float32))
    enable = int(np.asarray(enable_homogeneous_graph))
    if not enable:
        C = np.zeros_like(C)
        D = np.zeros_like(D)

    r = run_bass_kernel_spmd(_program(), _make_in_maps(C, A, D),
                             core_ids=list(range(NCORES))).results

    out = np.empty((N, N), dtype=np.float32)
    for c in range(NCORES):
        s = 512 * c
        e = min(s + CR, NMAT)
        n = e - s
        out[s:e, :] = r[c]["top"][:n]
        out[NMAT + s:NMAT + e, NMAT:] = r[c]["br"][:n]
        out[NMAT:, s:e] = r[c]["ats"][:, :n]
    idx = np.arange(N)
    out[idx, idx] += np.float32(1.0)
    return out


# revision 9
# speedup vs baseline: 1.5091x; 1.1337x over previous
"""Trainium2 Bass kernel for ConstructAdjMatrixWithHomogeneous — single launch.

out = I + D^-1/2 @ adj @ D^-1/2,  adj = [[C, A], [A^T, Dd]],
C = filtered_cell_kernel [4000,4000], Dd = filtered_drug_sim [4000,4000],
A = original_cell_drug_adj [4000,4000]; deg = rowsum(adj)+eps, d = deg**-0.5.

Sharding (8 cores): 128-aligned row bands. Core c owns rows
[512c, 512c+512) of each matrix; core 7 owns rows 3584:4000 plus 96
zero-padded junk rows so every DMA tile is a full [128,4000]
(partial-partition DMAs measured 3x slower per byte on this part).

Single launch per core, load order A -> D -> C so the drug-side
dependency chain resolves while C is still loading:
  - A bands: rowsums (DVE) + column-sum partials via bf16 ones-matmul
    (bf16 convert per 512-chunk; 4x faster PE than fp32, error ~1e-4
    on a 4000-term sum). Partials ReduceScatter(add) so each core gets
    its own 512-row drug-degree chunk back (~t=30us).
  - D bands: rowsums; drug deg = rs_d + RS chunk (PE-transposed to
    [128,4]); d = sqrt(reciprocal(deg+eps)) (ACT Rsqrt is banned);
    AllGather(drug d) fires ~t=50us.
  - C bands: rowsums; cell deg = rs_c + rs_a local; AllGather(cell d)
    fires ~t=72us right after the last load.
  Phase B (per 512-col chunk: PE ones-row matmul broadcasts the
  gathered d line into PSUM, then ONE fused in-place DVE op
  scalar_tensor_tensor(x, x, d_row, bc) = (x*d_row)*d_col):
  BR first (only needs drug d, ~t=60), then TR, then ats = PE
  transposes of the scaled A tiles, TL last (cell d arrives ~t=82).

Ring discipline (measured): one HWDGE ring sustains ~317 GB/s, two
directions on separate rings ~420 GB/s aggregate, mixed directions on
a ring much worse. Sync ring: all input loads, then BR + ats stores.
Scalar ring: collective bounce + d-chunk traffic, then TR + TL stores.

The +I is added on the host during assembly (O(N) work).
"""
import sys

sys.path.insert(0, "/opt/trn_rl_repo")

import json
import numpy as np

import concourse.bass as bass
import concourse.mybir as mybir
import concourse.tile as tile
import concourse.bass2jax as bass2jax
from concourse.alu_op_type import AluOpType
from concourse.bass_utils import run_bass_kernel_spmd, compile_bir_kernel

F32 = mybir.dt.float32
BF16 = mybir.dt.bfloat16
NCORES = 8
PB = 128               # partition band size
NBAND = 4
CR = PB * NBAND        # 512 rows of each matrix per core
NMAT = 4000
N = 8000
EPS = np.float32(1e-9)
NCH = 8                # 512-wide column chunks; last is 416
CHW = [512] * 7 + [416]
NFULL = NMAT // PB     # 31 full transpose chunks
TAIL = NMAT - NFULL * PB  # 32

# ---------------------------------------------------------------------------
# Walrus workaround: this toolchain only supports ONE sync-wait condition per
# instruction ("Too many sync wait commands" in CoreV3GenImpl otherwise).
# Split any instruction carrying >1 waits into preceding NoOps, 1 wait each.
# ---------------------------------------------------------------------------
_MAXW = 1


def _split_waits_bytes(bir_bytes):
    bir = json.loads(bir_bytes)
    n_new = 0
    for fn in bir["functions"]:
        for blk in fn["blocks"]:
            insts = blk.get("instructions", [])
            out = []
            for ins in insts:
                si = ins.get("sync_info") or {}
                waits = si.get("on_wait") or []
                while len(waits) > _MAXW:
                    chunk, waits = waits[:_MAXW], waits[_MAXW:]
                    n_new += 1
                    out.append({
                        "name": ins["name"] + f"_ws{n_new}",
                        "opcode": "NoOp",
                        "engine": ins["engine"],
                        "ins": [], "outs": [],
                        "sync_info": {"on_update": [], "on_wait": chunk},
                    })
                si["on_wait"] = waits
                ins["sync_info"] = si
                out.append(ins)
            blk["instructions"] = out
    return json.dumps(bir).encode()


def _patched_compile_bir_kernel(bir_json, tmpdir, neff_name="file.neff"):
    return compile_bir_kernel(_split_waits_bytes(bir_json), tmpdir,
                              neff_name=neff_name)


bass2jax.compile_bir_kernel = _patched_compile_bir_kernel


def _build(reps=1, no_coll=False, stage="full", timing_mode=False):
    nc = bass.Bass(num_devices=NCORES)
    cb = nc.declare_dram_parameter("cb", [CR, NMAT], F32, isOutput=False)
    ab = nc.declare_dram_parameter("ab", [CR, NMAT], F32, isOutput=False)
    db = nc.declare_dram_parameter("db", [CR, NMAT], F32, isOutput=False)
    if no_coll:
        rs_out_h = nc.declare_dram_parameter("rs_out_h", [1, 512], F32,
                                             isOutput=False)
        agc_out_h = nc.declare_dram_parameter("agc_out_h", [1, 4096], F32,
                                              isOutput=False)
        agd_out_h = nc.declare_dram_parameter("agd_out_h", [1, 4096], F32,
                                              isOutput=False)
    if timing_mode:
        # Identical device work, but big results land in Internal DRAM
        # scratch so the PJRT output plumbing (which costs ~0.5ms/MB per
        # call and jitters) stays tiny. One real [1,1] output remains.
        ok = nc.declare_dram_parameter("ok", [1, 1], F32, isOutput=True)
    else:
        top = nc.declare_dram_parameter("top", [CR, N], F32, isOutput=True)
        br = nc.declare_dram_parameter("br", [CR, NMAT], F32, isOutput=True)
        ats = nc.declare_dram_parameter("ats", [NMAT, CR], F32, isOutput=True)

    ident = nc.inline_tensor(np.eye(PB, dtype=np.float32), name="ident128")
    Sqrt = mybir.ActivationFunctionType.Sqrt
    RG = [list(range(NCORES))]

    with tile.TileContext(nc) as tc:
        with (
            tc.tile_pool(name="const", bufs=1) as const,
            tc.tile_pool(name="data", bufs=1) as data,
            tc.tile_pool(name="small", bufs=1) as small,
            tc.tile_pool(name="stg", bufs=2) as stg,
            tc.tile_pool(name="dstg", bufs=2) as dstg,
            tc.tile_pool(name="att", bufs=2) as att,
            tc.tile_pool(name="bfp", bufs=2) as bfp,
            tc.tile_pool(name="ps", bufs=1, space="PSUM") as ps,
            tc.tile_pool(name="dram", bufs=1, space="DRAM") as dram,
        ):
            idt = const.tile([PB, PB], F32, name="idt")
            nc.sync.dma_start(idt[:], ident[:])
            ones_bf = const.tile([PB, 1], BF16, name="ones_bf")
            nc.gpsimd.memset(ones_bf[:], 1.0)
            ones_row = const.tile([1, PB], F32, name="ones_row")
            nc.gpsimd.memset(ones_row[:], 1.0)
            ones1 = const.tile([1, 1], F32, name="ones1")
            nc.gpsimd.memset(ones1[:], 1.0)
            if timing_mode:
                top = dram.tile([CR, N], F32, tag="top_s", name="top_s")
                br = dram.tile([CR, NMAT], F32, tag="br_s", name="br_s")
                ats = dram.tile([NMAT, CR], F32, tag="ats_s", name="ats_s")

            for _ in range(reps):
                rs_in = dram.tile([1, 4096], F32, tag="rs_in", name="rs_in")
                rs_out = dram.tile([1, 512], F32, tag="rs_out", name="rs_out")
                agc_in = dram.tile([1, 512], F32, tag="agc_in", name="agc_in")
                agc_out = dram.tile([1, 4096], F32, tag="agc_out", name="agc_out")
                agd_in = dram.tile([1, 512], F32, tag="agd_in", name="agd_in")
                agd_out = dram.tile([1, 4096], F32, tag="agd_out", name="agd_out")

                rs_a = small.tile([PB, NBAND], F32, tag="rs_a", name="rs_a")
                rs_c = small.tile([PB, NBAND], F32, tag="rs_c", name="rs_c")
                rs_d = small.tile([PB, NBAND], F32, tag="rs_d", name="rs_d")
                deg_c = small.tile([PB, NBAND], F32, tag="deg_c", name="deg_c")
                deg_d = small.tile([PB, NBAND], F32, tag="deg_d", name="deg_d")
                drow_c = small.tile([PB, NBAND], F32, tag="drow_c", name="drow_c")
                drow_d = small.tile([PB, NBAND], F32, tag="drow_d", name="drow_d")

                # ---- A bands: rowsums + bf16 colsum partials ----
                cs_ps = [ps.tile([1, 512], F32, tag=f"ps{j}", name=f"cs{j}")
                         for j in range(NCH)]
                ta = []
                for b in range(NBAND):
                    t = data.tile([PB, NMAT], F32, tag=f"a{b}", name="ta")
                    nc.sync.dma_start(t[:], ab[b * PB:(b + 1) * PB, :])
                    nc.vector.reduce_sum(rs_a[:, b:b + 1], t[:],
                                         axis=mybir.AxisListType.X)
                    for j in range(NCH):
                        w = CHW[j]
                        xbf = bfp.tile([PB, 512], BF16, tag="bf", name="xbf")
                        nc.vector.tensor_copy(xbf[:, :w],
                                              t[:, 512 * j:512 * j + w])
                        nc.tensor.matmul(cs_ps[j][:, :w], ones_bf[:],
                                         xbf[:, :w],
                                         start=(b == 0), stop=(b == NBAND - 1))
                    ta.append(t)

                for j in range(NCH):
                    w = CHW[j]
                    cst = stg.tile([1, 512], F32, tag="cs_stg", name="cst")
                    nc.scalar.copy(cst[:, :w], cs_ps[j][:, :w])
                    nc.scalar.dma_start(rs_in[0:1, 512 * j:512 * j + w],
                                        cst[:, :w])
                if no_coll:
                    rs_out = rs_out_h
                else:
                    nc.gpsimd.collective_compute(
                        "ReduceScatter", AluOpType.add, replica_groups=RG,
                        ins=[rs_in.opt()], outs=[rs_out.opt()])

                # ---- D bands: rowsums -> drug d -> AllGather(drug) ----
                td = []
                for b in range(NBAND):
                    t = data.tile([PB, NMAT], F32, tag=f"d{b}", name="td")
                    nc.sync.dma_start(t[:], db[b * PB:(b + 1) * PB, :])
                    nc.vector.reduce_sum(rs_d[:, b:b + 1], t[:],
                                         axis=mybir.AxisListType.X)
                    td.append(t)
                rso = stg.tile([1, 512], F32, tag="rso", bufs=1, name="rso")
                nc.scalar.dma_start(rso[:], rs_out[:])
                ptcs = ps.tile([PB, NBAND], F32, tag="ps0", name="ptcs")
                for b in range(NBAND):
                    nc.tensor.transpose(ptcs[:, b:b + 1],
                                        rso[0:1, PB * b:PB * (b + 1)],
                                        ones1[:])
                nc.vector.tensor_add(deg_d[:], rs_d[:], ptcs[:])
                nc.vector.tensor_scalar_add(deg_d[:], deg_d[:], float(EPS))
                nc.vector.reciprocal(drow_d[:], deg_d[:])
                nc.scalar.activation(drow_d[:], drow_d[:], Sqrt)
                ptd = ps.tile([1, 512], F32, tag="ps1", name="ptd")
                for b in range(NBAND):
                    nc.tensor.transpose(ptd[0:1, PB * b:PB * (b + 1)],
                                        drow_d[:, b:b + 1], idt[:])
                agds = stg.tile([1, 512], F32, tag="ag_stg", bufs=1, name="agds")
                nc.scalar.copy(agds[:], ptd[:])
                nc.scalar.dma_start(agd_in[:], agds[:])
                if no_coll:
                    agd_out = agd_out_h
                else:
                    nc.gpsimd.collective_compute(
                        "AllGather", AluOpType.bypass, replica_groups=RG,
                        ins=[agd_in.opt()], outs=[agd_out.opt()])

                # ---- C bands: rowsums -> cell d -> AllGather(cell) ----
                tcl = []
                for b in range(NBAND):
                    t = data.tile([PB, NMAT], F32, tag=f"c{b}", name="tcl")
                    nc.sync.dma_start(t[:], cb[b * PB:(b + 1) * PB, :])
                    nc.vector.reduce_sum(rs_c[:, b:b + 1], t[:],
                                         axis=mybir.AxisListType.X)
                    tcl.append(t)
                nc.vector.tensor_add(deg_c[:], rs_c[:], rs_a[:])
                nc.vector.tensor_scalar_add(deg_c[:], deg_c[:], float(EPS))
                nc.vector.reciprocal(drow_c[:], deg_c[:])
                nc.scalar.activation(drow_c[:], drow_c[:], Sqrt)
                ptc = ps.tile([1, 512], F32, tag="ps2", name="ptc")
                for b in range(NBAND):
                    nc.tensor.transpose(ptc[0:1, PB * b:PB * (b + 1)],
                                        drow_c[:, b:b + 1], idt[:])
                agcs = stg.tile([1, 512], F32, tag="ag_stg", bufs=1, name="agcs")
                nc.scalar.copy(agcs[:], ptc[:])
                nc.scalar.dma_start(agc_in[:], agcs[:])
                if no_coll:
                    agc_out = agc_out_h
                else:
                    nc.gpsimd.collective_compute(
                        "AllGather", AluOpType.bypass, replica_groups=RG,
                        ins=[agc_in.opt()], outs=[agc_out.opt()])

                if stage == "phaseA":
                    continue

                # ---- phase B. BR first (drug d only, earliest ready) ----
                for j in range(NCH):
                    w = CHW[j]
                    sl = slice(512 * j, 512 * j + w)
                    dch = dstg.tile([1, 512], F32, tag="dch", name="dch")
                    nc.scalar.dma_start(dch[:, :w], agd_out[0:1, sl])
                    bc = ps.tile([PB, 512], F32, tag=f"ps{5 + (j % 2)}",
                                 name="bc")
                    nc.tensor.matmul(bc[:, :w], ones_row[:], dch[0:1, :w],
                                     start=True, stop=True)
                    for b in range(NBAND):
                        nc.vector.scalar_tensor_tensor(
                            td[b][:, sl], td[b][:, sl], drow_d[:, b:b + 1],
                            bc[:, :w], AluOpType.mult, AluOpType.mult)
                for b in range(NBAND):
                    nc.sync.dma_start(br[b * PB:(b + 1) * PB, :], td[b][:])

                # ---- TR = dcell * A * ddrug ----
                for j in range(NCH):
                    w = CHW[j]
                    sl = slice(512 * j, 512 * j + w)
                    dch = dstg.tile([1, 512], F32, tag="dch", name="dch")
                    nc.scalar.dma_start(dch[:, :w], agd_out[0:1, sl])
                    bc = ps.tile([PB, 512], F32, tag=f"ps{3 + (j % 2)}",
                                 name="bct")
                    nc.tensor.matmul(bc[:, :w], ones_row[:], dch[0:1, :w],
                                     start=True, stop=True)
                    for b in range(NBAND):
                        nc.vector.scalar_tensor_tensor(
                            ta[b][:, sl], ta[b][:, sl], drow_c[:, b:b + 1],
                            bc[:, :w], AluOpType.mult, AluOpType.mult)
                for b in range(NBAND):
                    nc.scalar.dma_start(top[b * PB:(b + 1) * PB, NMAT:],
                                        ta[b][:])

                # ---- TL = dcell * C * dcell ----
                for j in range(NCH):
                    w = CHW[j]
                    sl = slice(512 * j, 512 * j + w)
                    dch = dstg.tile([1, 512], F32, tag="dch", name="dch")
                    nc.scalar.dma_start(dch[:, :w], agc_out[0:1, sl])
                    bc = ps.tile([PB, 512], F32, tag=f"ps{5 + (j % 2)}",
                                 name="bcc")
                    nc.tensor.matmul(bc[:, :w], ones_row[:], dch[0:1, :w],
                                     start=True, stop=True)
                    for b in range(NBAND):
                        nc.vector.scalar_tensor_tensor(
                            tcl[b][:, sl], tcl[b][:, sl], drow_c[:, b:b + 1],
                            bc[:, :w], AluOpType.mult, AluOpType.mult)

                if stage == "scale":
                    for b in range(NBAND):
                        nc.scalar.dma_start(top[b * PB:(b + 1) * PB, 0:NMAT],
                                            tcl[b][:])
                    continue

                # ---- ats = (scaled A)^T; TL stores last on scalar ----
                for c in range(NFULL + 1):
                    cw = PB if c < NFULL else TAIL
                    pt = ps.tile([PB, 512], F32,
                                 tag=("ps7", "ps0", "ps1")[c % 3], name="pt")
                    for b in range(NBAND):
                        nc.tensor.transpose(
                            pt[:cw, b * PB:(b + 1) * PB],
                            ta[b][:, c * PB:c * PB + cw], idt[:])
                    at_sb = att.tile([PB, 512], F32, tag="att", name="at_sb")
                    nc.scalar.copy(at_sb[:cw, :], pt[:cw, :])
                    nc.sync.dma_start(ats[c * PB:c * PB + cw, :],
                                      at_sb[:cw, :])
                for b in range(NBAND):
                    nc.scalar.dma_start(top[b * PB:(b + 1) * PB, 0:NMAT],
                                        tcl[b][:])
                if timing_mode:
                    nc.scalar.dma_start(ok[:], drow_c[0:1, 0:1])
    return nc


_programs_cache = {}


def _program():
    if "l" not in _programs_cache:
        _programs_cache["l"] = _build()
    return _programs_cache["l"]


def _make_in_maps(C, A, D):
    in_maps = []
    for c in range(NCORES):
        s = 512 * c
        e = min(s + CR, NMAT)
        if e - s == CR:
            in_maps.append({"cb": C[s:e], "ab": A[s:e], "db": D[s:e]})
        else:
            m = {}
            for name, M in (("cb", C), ("ab", A), ("db", D)):
                t = np.zeros((CR, NMAT), dtype=np.float32)
                t[:e - s] = M[s:e]
                m[name] = t
            in_maps.append(m)
    return in_maps


def kernel(filtered_cell_kernel, filtered_drug_sim, original_cell_drug_adj,
           enable_homogeneous_graph):
    C = np.ascontiguousarray(np.asarray(filtered_cell_kernel, dtype=np.float32))
    D = np.ascontiguousarray(np.asarray(filtered_drug_sim, dtype=np.float32))
    A = np.ascontiguousarray(np.asarray(original_cell_drug_adj, dtype=np.float32))
    enable = int(np.asarray(enable_homogeneous_graph))
    if not enable:
        C = np.zeros_like(C)
        D = np.zeros_like(D)

    r = run_bass_kernel_spmd(_program(), _make_in_maps(C, A, D),
                             core_ids=list(range(NCORES))).results

    out = np.empty((N, N), dtype=np.float32)
    for c in range(NCORES):
        s = 512 * c
        e = min(s + CR, NMAT)
        n = e - s
        out[s:e, :] = r[c]["top"][:n]
        out[NMAT + s:NMAT + e, NMAT:] = r[c]["br"][:n]
        out[NMAT:, s:e] = r[c]["ats"][:, :n]
    idx = np.arange(N)
    out[idx, idx] += np.float32(1.0)
    return out
